# revision 18
# baseline (speedup 1.0000x reference)
"""AdaptiveFractalAnalysis distributed Trainium2 kernel (8 NeuronCores).

Strategy
--------
The reference computes three "fractal dimension" statistics of x [8192, 256]:
  - box-counting: pooled = avg_pool(x, s); count(pooled > pooled.mean()) per scale
  - correlation:  count(pairwise_dist(x) < s)  -> dominated by an 8192x8192x256 matmul
  - information:  histogram entropy of x per scale
then host-side slope fits and a softmax-weighted sum (scalar output).

Device split (uniform SPMD graph on 8 cores, no collectives -- final tiny
reduction happens on host):
  - cdist: d2 = sq_i + sq_j - 2 x@x.T. Using symmetry, the 16x16 grid of
    512-row blocks is covered once per unordered pair (136 pairs = 8 cores x 17).
    Per core the pairs are organized into "runs" sharing the lhs block so one
    PSUM group holds [128, 512*len(run)] and threshold counting amortizes.
    PSUM holds v = x@x.T - 0.5*sq_j (bf16 matmuls; sq_j via a K=2 ones-row
    matmul with bf16 hi/lo split). Count(d2 < t) == count(v > (sq_i - t)/2),
    per-partition thresholds. Counting runs on DVE (custom 2-threshold op,
    base-4096 packed exact counts) and ScalarE (Sign activation with
    per-partition bias + fused accumulation), greedily balanced.
  - box: pooled values for all scales computed transposed via matmul with a
    block-pooling matrix (partition = pooled column, free = row index), then
    one Sign-activation count per PSUM group with per-partition -theta bias.
  - hist: cumulative counts count(x < edge) for the deduped interior bin
    edges, on the core's own rows (f32, exact), split DVE/ACT.
Each counting instruction writes a per-partition accumulator into a column of
an SBUF "acc" tile; acc is DMA'd out and all decoding/slope math is numpy.
"""

import sys
import numpy as np

if "/opt/trn_rl_repo" not in sys.path:
    sys.path.insert(0, "/opt/trn_rl_repo")

import ml_dtypes

bf16 = ml_dtypes.bfloat16


N_ROWS, DIM = 8192, 256
NBLK = 16            # 512-row blocks
BLK = 512
NCORES = 8
B_PACK = 4096.0      # exact-int packing base for the 2-threshold DVE op
BIG = 3.0e38         # sentinel threshold: count(v > BIG) == 0

_BUILD_CACHE = {}
_CNT2 = None
_CNT2S = None


def _patch_ldw_opt():
    """walrus ldw-opt dedupes back-to-back LDWEIGHTS sharing a stationary
    operand (and enables FWL); concourse disables it by default."""
    import concourse.bass_utils as _bu
    if getattr(_bu, "_afa_ldw_patched", False):
        return
    _orig = _bu.run_command

    def _patched(cmd, *a, **kw):
        try:
            cmd = ["--enable-ldw-opt=true" if c == "--enable-ldw-opt=false"
                   else c for c in cmd]
        except TypeError:
            pass
        return _orig(cmd, *a, **kw)

    _bu.run_command = _patched
    _bu._afa_ldw_patched = True


# _patch_ldw_opt()  # walrus rejects our LDW pattern


# --------------------------------------------------------------------------
# custom DVE op: out = (x > c0) + (x > c1)*B ; accum_out = sum(out)
# --------------------------------------------------------------------------
def _register_cnt2():
    global _CNT2
    if _CNT2 is not None:
        return _CNT2
    import operator
    from concourse import dve_ops
    from concourse.dve_spec import Spec, Src0, C0, C1, C2, lower, _has_src1
    from concourse.dve_uop import DveOpSpec

    name = "CNT2_ANT_AFA"
    for o in dve_ops.OPS:
        if o.name == name:
            _CNT2 = o
            return o
    spec = Spec(
        body=(Src0 > C0) + (Src0 > C1) * C2,
        accum=operator.add,
        reference=lambda in0, in1, s0, s1, imm2: (
            (in0 > s0).astype(np.float32) + (in0 > s1).astype(np.float32) * imm2
        ),
    )
    row = dve_ops._CUSTOM_DVE_ROW_BASE + len(dve_ops.OPS)
    assert row < 0x20
    dve_ops._SUB_OPCODE_FOR_NAME[name] = row
    shas = {}
    for ver in ("v3",):
        uops = lower(spec, ver=ver)
        tmp = DveOpSpec(name=name, opcode=row, uops=uops, rd1_en=_has_src1(spec))
        shas[ver] = tmp.sha(ver)
    op = dve_ops.DveOp(name, spec, subdim=False, uops_sha=shas)
    dve_ops.OPS.append(op)
    dve_ops.CUSTOM_DVE_SPECS[name] = spec
    _CNT2 = op
    return op


def _register_cnt2s():
    """out = ((x - y) > c0) + ((x - y) > c1)*B ; accum_out = sum(out).
    y (Src1) carries 0.5*sq_j so the PE never has to add it into PSUM."""
    global _CNT2S
    if _CNT2S is not None:
        return _CNT2S
    import operator
    from concourse import dve_ops
    from concourse.dve_spec import Spec, Src0, Src1, C0, C1, C2, lower, _has_src1
    from concourse.dve_uop import DveOpSpec

    name = "CNT2S_ANT_AFA"
    for o in dve_ops.OPS:
        if o.name == name:
            _CNT2S = o
            return o
    u_ = Src0 - Src1
    spec = Spec(
        body=(u_ > C0) + (u_ > C1) * C2,
        accum=operator.add,
        reference=lambda in0, in1, s0, s1, imm2: (
            ((in0 - in1) > s0).astype(np.float32)
            + ((in0 - in1) > s1).astype(np.float32) * imm2
        ),
    )
    row = dve_ops._CUSTOM_DVE_ROW_BASE + len(dve_ops.OPS)
    assert row < 0x20
    dve_ops._SUB_OPCODE_FOR_NAME[name] = row
    shas = {}
    for ver in ("v3",):
        uops = lower(spec, ver=ver)
        tmp = DveOpSpec(name=name, opcode=row, uops=uops, rd1_en=_has_src1(spec))
        shas[ver] = tmp.sha(ver)
    op = dve_ops.DveOp(name, spec, subdim=False, uops_sha=shas)
    dve_ops.OPS.append(op)
    dve_ops.CUSTOM_DVE_SPECS[name] = spec
    _CNT2S = op
    return op


# --------------------------------------------------------------------------
# pair assignment: cover all unordered block pairs, uniform per-core shape
# --------------------------------------------------------------------------
def _plan_runs():
    """Partition the 136 unordered block pairs into per-core runs.

    Every core gets the same run-length structure:
      offdiag runs of lengths OFF_STRUCT (pairs sharing the lhs block)
      + 2 diagonal single runs.
    Returns runs_per_core: list (len 8) of list of (a, [b...], is_diag).
    """
    # offdiag pairs per lhs row a: b in a+1..15 -> length 15-a
    OFF_STRUCT = (4, 4, 4, 2, 1)          # 15 offdiag pairs per core
    need = {4: 0, 2: 0, 1: 0}
    for s in OFF_STRUCT:
        need[s] += NCORES
    # cut rows (lengths 15,14,...,0) into chunks from the multiset `need`
    rows = [(a, list(range(a + 1, NBLK))) for a in range(NBLK)]
    chunks = {4: [], 2: [], 1: []}
    # greedy with small backtracking: take largest needed chunk that fits
    rows_sorted = sorted(rows, key=lambda r: -len(r[1]))
    for a, bs in rows_sorted:
        i = 0
        rem = bs
        while rem:
            for size in (4, 2, 1):
                if len(chunks[size]) < need[size] and len(rem) >= size:
                    chunks[size].append((a, rem[:size]))
                    rem = rem[size:]
                    break
            else:
                # force split into singles if mismatch (shouldn't happen with
                # counts below, but keep safe)
                chunks[1].append((a, rem[:1]))
                rem = rem[1:]
    ok = all(len(chunks[s]) == need[s] for s in (4, 2, 1))
    if not ok:
        # fallback: all doubles + singles structure (always feasible)
        OFF_STRUCT = (2, 2, 2, 2, 2, 2, 2, 1)
        need = {4: 0, 2: 0, 1: 0}
        for s in OFF_STRUCT:
            need[s] += NCORES
        chunks = {4: [], 2: [], 1: []}
        for a, bs in rows_sorted:
            rem = list(bs)
            while rem:
                for size in (2, 1):
                    if len(chunks[size]) < need[size] and len(rem) >= size:
                        chunks[size].append((a, rem[:size]))
                        rem = rem[size:]
                        break
                else:
                    chunks[1].append((a, rem[:1]))
                    rem = rem[1:]
        assert all(len(chunks[s]) == need[s] for s in (4, 2, 1)), (
            {k: len(v) for k, v in chunks.items()})
    runs_per_core = []
    for c in range(NCORES):
        runs = []
        for s in OFF_STRUCT:
            a, bs = chunks[s].pop()
            runs.append((a, bs, False))
        runs.append((2 * c, [2 * c], True))
        runs.append((2 * c + 1, [2 * c + 1], True))
        runs_per_core.append(runs)
    return OFF_STRUCT, runs_per_core


# --------------------------------------------------------------------------
# build the bass kernel for a given (u, E, box-structure) config
# --------------------------------------------------------------------------
def _build(cfg_key, u, n_tp, edges, box_groups, run_struct):
    """run_struct: tuple of run lengths incl 2 diag singles, same every core.
    box_groups: list of group sizes (#pooled columns per PSUM group), <=128.
    Returns (nc, meta) where meta describes acc slot layout.
    """
    from concourse import bacc, tile, mybir

    CNT2 = _register_cnt2()
    CNT2S = _register_cnt2s()
    f32 = mybir.dt.float32
    bt = mybir.dt.bfloat16
    AT = mybir.ActivationFunctionType
    ALU = mybir.AluOpType

    n_runs = len(run_struct)
    n_pairs = sum(run_struct)
    assert n_pairs == 17
    E = len(edges)
    NG = len(box_groups)
    MTOT = sum(box_groups)

    # ---- engine cost model (ns) for balancing count passes ----
    def dve_cost(w):      # CNT2 (2 thresholds) over [128, w] psum/sbuf 1x
        return (225 + w) / 0.96

    def act_cost(w):      # Sign+accum (1 threshold)
        return (180 + w) / 1.2 + 185

    # Per group, columns [0, F) are counted by DVE (CNT2S with the sq_j
    # broadcast as Src1 -- no PSUM sq_j needed) and [F, w) by ACT Sign
    # (which needs the K=2 nsq matmul on its columns). F is 512-aligned so
    # the nsq matmuls stay within PSUM banks. Chosen greedily to balance
    # cumulative engine loads (cost model below).
    cum = {"dve": 0.0, "act": 0.0}   # seeded below after hist/box planning

    def choose_F(w):
        # F on a 512 grid (nsq-matmul bank alignment). Objective trades off
        # cumulative engine balance against per-group wall time (a group must
        # be fully counted before its PSUM slot is reused; a lumpy
        # single-engine group stalls the PE long enough to re-throttle HAM).
        best = None
        for F in range(0, w + 1, 512):
            d = n_tp * dve_cost(F) if F else 0.0
            a = u * act_cost(w - F) if w - F else 0.0
            score = max(cum["dve"] + d, cum["act"] + a) + 0.7 * max(d, a)
            if best is None or score < best[0]:
                best = (score, F, d, a)
        _, F, d, a = best
        cum["dve"] += d
        cum["act"] += a
        return F

    nc = bacc.Bacc("TRN2", target_bir_lowering=False, debug=False,
                   num_devices=NCORES)
    dL = nc.dram_tensor("L", [2, n_runs, 128, BLK], bt, kind="ExternalInput")
    dR = nc.dram_tensor("R", [2, n_runs, 128, 2048], bt, kind="ExternalInput")
    dNSQ = nc.dram_tensor("NSQ", [n_runs, 128, 2048], bt, kind="ExternalInput")
    dNSQB = nc.dram_tensor("NSQB", [n_runs, 128, 2048], bt, kind="ExternalInput")
    dCIK = nc.dram_tensor("CIK", [128, n_runs * 4 * u], f32, kind="ExternalInput")
    dXF = nc.dram_tensor("XF", [128, 2048], f32, kind="ExternalInput")
    dBX = nc.dram_tensor("BX", [2, 128, 1024], bt, kind="ExternalInput")
    dPM = nc.dram_tensor("PM", [2, 128, max(MTOT, 1)], bt, kind="ExternalInput")
    dBTH = nc.dram_tensor("BTH", [128, max(NG, 1)], f32, kind="ExternalInput")
    dHED = nc.dram_tensor("HED", [128, max(E, 1)], f32, kind="ExternalInput")
    NSLOT = 512
    dOUT = nc.dram_tensor("OUT", [128, NSLOT], f32, kind="ExternalOutput")

    meta = {"cdist": [], "box": [], "hist": []}
    slot_ctr = [0]

    def new_slot():
        sl = slot_ctr[0]
        slot_ctr[0] += 1
        assert sl < NSLOT
        return sl

    # threshold pairs
    tps = []
    k = 0
    while k < u:
        tps.append((k, k + 1) if k + 1 < u else (k, None))
        k += 2
    assert len(tps) == n_tp

    # ---- hist work queue (ops on xf [128,2048]), balanced by cost ----
    hist_queue = []    # ("dve", ea, eb) or ("act", ea)
    hl = {"dve": 0.0, "act": 0.0}
    ei = 0
    while ei < E:
        if ei + 1 < E and hl["dve"] + dve_cost(2048) <= hl["act"] + 2 * act_cost(2048):
            hist_queue.append(("dve", ei, ei + 1))
            hl["dve"] += dve_cost(2048)
            ei += 2
        elif hl["act"] + act_cost(2048) <= hl["dve"] + dve_cost(2048):
            hist_queue.append(("act", ei, None))
            hl["act"] += act_cost(2048)
            ei += 1
        else:
            hist_queue.append(("dve", ei, None))
            hl["dve"] += dve_cost(2048)
            ei += 1

    # seed the balancer with hist + box loads (emitted on these engines)
    cum["dve"] += sum(dve_cost(2048) for q in hist_queue if q[0] == "dve")
    cum["act"] += sum(act_cost(2048) for q in hist_queue if q[0] == "act")
    cum["act"] += NG * act_cost(1024)

    with tile.TileContext(nc) as tc:
        import contextlib
        ctx = contextlib.ExitStack()
        with ctx:
            const_p = ctx.enter_context(tc.tile_pool(name="const", bufs=1))
            acc = const_p.tile([128, NSLOT], f32)
            nc.vector.memset(acc[:], 0.0)

            lp = ctx.enter_context(tc.tile_pool(name="lp", bufs=len(run_struct)))
            rp = ctx.enter_context(tc.tile_pool(name="rp", bufs=len(run_struct)))
            np_ = ctx.enter_context(tc.tile_pool(name="nsqp", bufs=len(run_struct)))

            # prefetch all run inputs first (block-granular R so the first
            # matmuls start as soon as their slices land)
            run_tiles = []
            for ri, rl in enumerate(run_struct):
                w = rl * BLK
                l0 = lp.tile([128, BLK], bt, tag="l0")
                nc.gpsimd.dma_start(l0[:], dL[0, ri])
                l1 = lp.tile([128, BLK], bt, tag="l1")
                nc.sync.dma_start(l1[:], dL[1, ri])
                r0 = rp.tile([128, 2048], bt, tag="r0")
                r1 = rp.tile([128, 2048], bt, tag="r1")
                for j in range(rl):
                    nc.gpsimd.dma_start(r0[:, j * BLK:(j + 1) * BLK],
                                        dR[0, ri, :, j * BLK:(j + 1) * BLK])
                    nc.sync.dma_start(r1[:, j * BLK:(j + 1) * BLK],
                                      dR[1, ri, :, j * BLK:(j + 1) * BLK])
                nst = np_.tile([128, 2048], bt, tag="nst")
                nc.gpsimd.dma_start(nst[:, 0:w], dNSQ[ri, :, 0:w])
                nsqb = rp.tile([128, 2048], bt, tag="nsqb")
                for j in range(rl):
                    nc.sync.dma_start(nsqb[:, j * BLK:(j + 1) * BLK],
                                      dNSQB[ri, :, j * BLK:(j + 1) * BLK])
                run_tiles.append((l0, l1, r0, r1, nst, nsqb))

            cik = const_p.tile([128, n_runs * 4 * u], f32)
            nc.scalar.dma_start(cik[:], dCIK[:])
            xf = const_p.tile([128, 2048], f32)
            nc.scalar.dma_start(xf[:], dXF[:])
            hed = const_p.tile([128, max(E, 1)], f32)
            nc.scalar.dma_start(hed[:], dHED[:])
            bth = const_p.tile([128, max(NG, 1)], f32)
            nc.scalar.dma_start(bth[:], dBTH[:])
            ones2 = const_p.tile([128, 128], bt)
            nc.vector.memset(ones2[:], 0.0)
            nc.vector.memset(ones2[0:2, :], 1.0)
            wrm = const_p.tile([128, 512], bt)
            nc.vector.memset(wrm[:], 0.0)
            scr = const_p.tile([128, 2048], bt)     # dve scratch out
            scrf = const_p.tile([128, 2048], f32)   # act scratch out
            bx0 = const_p.tile([128, 1024], bt)
            nc.scalar.dma_start(bx0[:], dBX[0])
            bx1 = const_p.tile([128, 1024], bt)
            nc.scalar.dma_start(bx1[:], dBX[1])
            if MTOT > 0:
                pm0 = const_p.tile([128, MTOT], bt)
                nc.scalar.dma_start(pm0[:], dPM[0, :, 0:MTOT])
                pm1 = const_p.tile([128, MTOT], bt)
                nc.scalar.dma_start(pm1[:], dPM[1, :, 0:MTOT])

            def emit_hist_one():
                if not hist_queue:
                    return
                kind, ea, eb = hist_queue.pop(0)
                for half in range(2):
                    c0, c1 = half * 1024, (half + 1) * 1024
                    slot = new_slot()
                    if kind == "dve":
                        s1v = hed[:, eb:eb + 1] if eb is not None else BIG
                        nc.vector._custom_dve(
                            CNT2, out=scr[:, 0:1024], in0=xf[:, c0:c1],
                            s0=hed[:, ea:ea + 1], s1=s1v,
                            imm2=B_PACK, accum_out=acc[:, slot:slot + 1])
                    else:
                        nc.scalar.activation(
                            scrf[:, 0:1024], xf[:, c0:c1], AT.Sign,
                            bias=hed[:, ea:ea + 1], scale=-1.0,
                            accum_out=acc[:, slot:slot + 1])
                    meta["hist"].append((kind, slot, ea, eb, 1024))

            def emit_box():
                g0 = 0
                for g, mg in enumerate(box_groups):
                    pg = psum_p.tile([128, 2048], f32, tag="pg")
                    for nsl in range(2):
                        nc.tensor.matmul(
                            pg[0:mg, nsl * 512:(nsl + 1) * 512],
                            pm0[:, g0:g0 + mg],
                            bx0[:, nsl * 512:(nsl + 1) * 512],
                            start=True, stop=False)
                        nc.tensor.matmul(
                            pg[0:mg, nsl * 512:(nsl + 1) * 512],
                            pm1[:, g0:g0 + mg],
                            bx1[:, nsl * 512:(nsl + 1) * 512],
                            start=False, stop=True)
                    slot = new_slot()
                    # count(pooled > theta): sign(theta-pooled) -> (w - sum)/2
                    nc.scalar.activation(
                        scrf[0:mg, 0:1024], pg[0:mg, 0:1024], AT.Sign,
                        bias=bth[0:mg, g:g + 1], scale=-1.0,
                        accum_out=acc[0:mg, slot:slot + 1])
                    meta["box"].append((slot, g, mg, 1024))
                    g0 += mg

            # ---- PE warmup: dense matmul burst while input DMAs land.
            # A fully-busy ~4us window fires the HAM un-throttle (1.2->2.4GHz)
            # before the real matmul stream begins.
            with tc.tile_pool(name="wps", bufs=1, space="PSUM") as wps:
                wpt = wps.tile([128, 512], f32)
                for _ in range(24):
                    nc.tensor.matmul(wpt[:], ones2[:], wrm[:],
                                     start=True, stop=True)

            psum_p = ctx.enter_context(
                tc.tile_pool(name="cps", bufs=2, space="PSUM"))

            # ---- cdist runs ----
            pair_slot = 0
            for ri, rl in enumerate(run_struct):
                w = rl * BLK
                if ri == 1 and MTOT > 0:
                    emit_box()
                l0, l1, r0, r1, nst, nsqb = run_tiles[ri]
                for r in range(4):
                    pg = psum_p.tile([128, 2048], f32, tag="pg")
                    F = choose_F(w)
                    for kt in range(2):
                        lt = (l0, l1)[kt]
                        rt = (r0, r1)[kt]
                        for j in range(rl):
                            nc.tensor.matmul(
                                pg[:, j * BLK:(j + 1) * BLK],
                                lt[:, r * 128:(r + 1) * 128],
                                rt[:, j * BLK:(j + 1) * BLK],
                                start=(kt == 0), stop=(kt == 1 and j * BLK < F))
                    for j in range(F // BLK, rl):
                        nc.tensor.matmul(
                            pg[:, j * BLK:(j + 1) * BLK],
                            ones2[:], nst[:, j * BLK:(j + 1) * BLK],
                            start=False, stop=True)
                    base = (ri * 4 + r) * u
                    if F > 0:
                        for (ka, kb) in tps:
                            slot = new_slot()
                            cb = cik[:, base + kb:base + kb + 1] \
                                if kb is not None else BIG
                            nc.vector._custom_dve(
                                CNT2S, out=scr[:, 0:F], in0=pg[:, 0:F],
                                in1=nsqb[:, 0:F],
                                s0=cik[:, base + ka:base + ka + 1],
                                s1=cb, imm2=B_PACK,
                                accum_out=acc[:, slot:slot + 1])
                            meta["cdist"].append(
                                ("dve", slot, ri, r, ka, kb, F))
                    if F < w:
                        for kk in range(u):
                            slot = new_slot()
                            nc.scalar.activation(
                                scrf[:, 0:w - F], pg[:, F:w], AT.Sign,
                                bias=cik[:, base + kk:base + kk + 1],
                                scale=-1.0,
                                accum_out=acc[:, slot:slot + 1])
                            meta["cdist"].append(
                                ("act", slot, ri, r, kk, None, w - F))
                    if ri >= 2:
                        emit_hist_one()
                pair_slot += rl

            while hist_queue:
                emit_hist_one()

            nc.sync.dma_start(dOUT[:], acc[:])

    nc.compile()
    return nc, meta


# --------------------------------------------------------------------------
# host orchestration
# --------------------------------------------------------------------------
def kernel(x, scale_params, scale_importance):
    from concourse.bass_utils import run_bass_kernel_spmd

    x = np.asarray(x, dtype=np.float32)
    scale_params = np.asarray(scale_params, dtype=np.float32)
    scale_importance = np.asarray(scale_importance, dtype=np.float32)
    n, d = x.shape
    assert (n, d) == (N_ROWS, DIM)

    x64 = x.astype(np.float64)
    # ---- dynamic scales (mirror reference host-side computation) ----
    s = np.exp(scale_params.astype(np.float64))
    std_factor = float(x64.std(ddof=1) / x64.mean())
    std_factor = min(max(std_factor, 0.5), 2.0)
    adj = np.clip(s * std_factor, 2.0, 16.0)
    scales = [int(v) for v in adj]
    log_s = np.log(np.asarray(scales, np.float32)).astype(np.float64)

    # ---- derived constants ----
    uniq_scales = sorted(set(scales))
    uniq_t = sorted(set(float(ss) * float(ss) for ss in scales))
    u = len(uniq_t)
    n_tp = (u + 1) // 2

    # box: theta per unique scale; pooling matrix columns
    box_cols = []   # list of (scale, block_index)
    thetas = {}
    for ss in uniq_scales:
        m = d // ss
        nn = m * ss
        thetas[ss] = float(x64[:, :nn].sum() / (n * nn))
        for b in range(m):
            box_cols.append((ss, b))
    MTOT = len(box_cols)
    box_groups = []
    rem = MTOT
    while rem > 0:
        g = min(128, rem)
        box_groups.append(g)
        rem -= g
    NG = len(box_groups)

    # hist: deduped interior edges (f32 linspace like jnp.histogram)
    xmin = float(x.min())
    xmax = float(x.max())
    edge_list = []      # deduped values
    edge_map = {}       # (scale, k) -> index into edge_list
    for ss in uniq_scales:
        ed = np.linspace(np.float32(xmin), np.float32(xmax), ss + 1,
                         dtype=np.float32)
        for kk in range(1, ss):
            v = float(ed[kk])
            if v not in edge_map:
                edge_map[v] = len(edge_list)
                edge_list.append(v)
            edge_map[(ss, kk)] = edge_map[v]
    E = len(edge_list)

    run_struct_off, runs_per_core = _plan_runs()
    run_struct = tuple(list(run_struct_off) + [1, 1])

    cfg_key = (u, n_tp, E, tuple(box_groups), run_struct, MTOT)
    if cfg_key not in _BUILD_CACHE:
        _BUILD_CACHE[cfg_key] = _build(
            cfg_key, u, n_tp, edge_list, box_groups, run_struct)
    nc, meta = _BUILD_CACHE[cfg_key]

    # ---- per-core inputs ----
    xb = x.astype(bf16)                       # [8192, 256]
    xTb = np.ascontiguousarray(xb.T)          # [256, 8192]
    sq = (x.astype(np.float32) ** 2).sum(axis=1, dtype=np.float32)  # [8192]
    nsq_half = -0.5 * sq
    nsq_hi = nsq_half.astype(bf16)
    nsq_lo = (nsq_half - nsq_hi.astype(np.float32)).astype(bf16)

    n_runs = len(run_struct)
    n_pairs = 17

    # pooling matrix [256, MTOT] bf16 (same all cores)
    PM = np.zeros((256, max(MTOT, 1)), np.float32)
    for col, (ss, b) in enumerate(box_cols):
        PM[b * ss:(b + 1) * ss, col] = 1.0 / ss
    PM_b = PM.astype(bf16)
    dPM = np.stack([PM_b[0:128], PM_b[128:256]])          # [2,128,MTOT]
    dBTH = np.zeros((128, max(NG, 1)), np.float32)
    g0 = 0
    for g, mg in enumerate(box_groups):
        for p in range(mg):
            ss, b = box_cols[g0 + p]
            dBTH[p, g] = thetas[ss]
        g0 += mg

    t_arr = np.asarray(uniq_t, np.float64)
    dHED_np = np.zeros((128, max(E, 1)), np.float32)
    for ei2, ev in enumerate(edge_list):
        dHED_np[:, ei2] = ev

    in_maps = []
    core_meta = []
    for c in range(NCORES):
        runs = runs_per_core[c]
        L = np.zeros((2, n_runs, 128, BLK), bf16)
        R = np.zeros((2, n_runs, 128, 2048), bf16)
        NSQ = np.zeros((n_runs, 128, 2048), bf16)
        NSQB = np.zeros((n_runs, 128, 2048), bf16)
        CIK = np.zeros((128, n_runs * 4 * u), np.float32)
        ps = 0
        pair_list = []
        for ri, (a, bs, is_diag) in enumerate(runs):
            for kt in range(2):
                L[kt, ri] = xTb[kt * 128:(kt + 1) * 128,
                                a * BLK:(a + 1) * BLK]
            for j, b in enumerate(bs):
                for kt in range(2):
                    R[kt, ri, :, j * BLK:(j + 1) * BLK] = xTb[
                        kt * 128:(kt + 1) * 128, b * BLK:(b + 1) * BLK]
                NSQ[ri, 0, j * BLK:(j + 1) * BLK] = nsq_hi[b * BLK:(b + 1) * BLK]
                NSQ[ri, 1, j * BLK:(j + 1) * BLK] = nsq_lo[b * BLK:(b + 1) * BLK]
                NSQB[ri, :, j * BLK:(j + 1) * BLK] = (
                    0.5 * sq[b * BLK:(b + 1) * BLK]).astype(bf16)[None, :]
            for r in range(4):
                i0 = a * BLK + r * 128
                sqi = sq[i0:i0 + 128].astype(np.float64)
                for kk in range(u):
                    CIK[:, (ri * 4 + r) * u + kk] = (
                        (sqi - t_arr[kk]) * 0.5).astype(np.float32)
            pair_list.append((a, list(bs), is_diag))
            ps += len(bs)
        rows = x[c * 1024:(c + 1) * 1024]                  # own rows
        XF = np.ascontiguousarray(
            rows.reshape(8, 128, 256).transpose(1, 0, 2).reshape(128, 2048))
        rowsT_b = xTb[:, c * 1024:(c + 1) * 1024]
        BX = np.stack([rowsT_b[0:128], rowsT_b[128:256]])  # [2,128,1024]
        in_maps.append({
            "L": L, "R": R, "NSQ": NSQ, "NSQB": NSQB, "CIK": CIK,
            "XF": np.ascontiguousarray(XF),
            "BX": np.ascontiguousarray(BX),
            "PM": dPM, "BTH": dBTH, "HED": dHED_np,
        })
        core_meta.append(pair_list)

    res = None
    last_err = None
    for attempt in range(4):
        try:
            res = run_bass_kernel_spmd(nc, in_maps, core_ids=list(range(NCORES)))
            break
        except Exception as e:  # transient NRT_EXEC_UNIT_UNRECOVERABLE etc.
            last_err = e
            import time as _t
            _t.sleep(3.0 * (attempt + 1))
    if res is None:
        raise last_err

    # ---- decode ----
    corr_counts = np.zeros(u, np.float64)
    box_counts = {ss: 0.0 for ss in uniq_scales}
    hist_cum = np.zeros(E, np.float64)

    for c in range(NCORES):
        out = res.results[c]["OUT"].astype(np.float64)   # [128, NSLOT]
        pair_list = core_meta[c]
        for ent in meta["cdist"]:
            kind, slot, ri, r, ka, kb, w = ent
            a, bs, is_diag = pair_list[ri]
            wt = 1.0 if is_diag else 2.0
            vals = out[:, slot]
            if kind == "dve":
                c1 = np.mod(vals, B_PACK)
                c2 = np.floor(vals / B_PACK)
                corr_counts[ka] += wt * c1.sum()
                if kb is not None:
                    corr_counts[kb] += wt * c2.sum()
            else:
                # count(v > c) = (w - sum_sign)/2 per partition
                corr_counts[ka] += wt * ((w - vals) / 2.0).sum()
        for (slot, g, mg, wbox) in meta["box"]:
            vals = out[0:mg, slot]
            cnt = (wbox - vals) / 2.0     # count(pooled > theta)
            gg0 = sum(box_groups[:g])
            for p in range(mg):
                ss, b = box_cols[gg0 + p]
                box_counts[ss] += cnt[p]
        for ent in meta["hist"]:
            kind, slot, ea, eb, wh = ent
            vals = out[:, slot]
            if kind == "dve":
                cgt1 = np.mod(vals, B_PACK).sum()
                cgt2 = np.floor(vals / B_PACK).sum()
                hist_cum[ea] += 2048 * 128 - cgt1   # count(x < e) = w - count(x > e)  (ties ~0)
                hist_cum[eb] += 2048 * 128 - cgt2
            else:
                hist_cum[ea] += ((wh + vals) / 2.0).sum()

    # ---- slope fits (host) ----
    def slope(xv, yv):
        xv = np.asarray(xv, np.float64)
        yv = np.asarray(yv, np.float64)
        xm = xv.mean()
        ym = yv.mean()
        dx = xv - xm
        with np.errstate(divide="ignore", invalid="ignore"):
            return float((dx * (yv - ym)).sum() / (dx * dx).sum())

    t_index = {t: i for i, t in enumerate(uniq_t)}
    corr_per_scale = np.array(
        [corr_counts[t_index[float(ss) * float(ss)]] for ss in scales])
    box_per_scale = np.array([box_counts[ss] for ss in scales])

    ents = []
    total = float(n * d)
    for ss in scales:
        cum = np.zeros(ss + 1, np.float64)
        cum[0] = 0.0
        cum[ss] = total
        for kk in range(1, ss):
            cum[kk] = hist_cum[edge_map[(ss, kk)]]
        hist = np.diff(cum)
        p = hist / total
        with np.errstate(divide="ignore", invalid="ignore"):
            ents.append(float(-(np.where(p > 0, p * np.log(
                np.where(p > 0, p, 1.0)), 0.0)).sum()))

    with np.errstate(divide="ignore", invalid="ignore"):
        box_dim = -slope(log_s, np.log(box_per_scale))
        corr_dim = slope(log_s, np.log(corr_per_scale))
    info_dim = slope(log_s, np.asarray(ents))

    # softmax in f32 like the reference
    si = scale_importance.astype(np.float64)
    w_ = np.exp(si - si.max())
    w_ = w_ / w_.sum()
    out_val = w_[0] * box_dim + w_[1] * corr_dim + w_[2] * info_dim
    return np.float32(out_val)
